# revision 14
# baseline (speedup 1.0000x reference)
#
# Trainium2 Bass kernel for nn_Attention_22 (4-quadrant channel attention).
#
# Mapping: 16 independent (quadrant, batch) items -> 8 cores, each core
# processes one quadrant x one batch-PAIR, packed as 128 SBUF partitions
# (item a = partitions 0..63, item b = 64..127).
#
# Math per item (C=64 channels, Hq x Wq = 192x192 quadrant):
#   LayerNorm over C (per pixel) -> 1x1 conv to 3C -> depthwise 3x3 ->
#   q,k,v -> per-head (8x8) channel gram softmax -> attn@v -> 1x1 proj ->
#   grw*x + out.
#
# Key kernel tricks:
#   * LN scale/shift + 1x1 conv + depthwise 3x3 are fused into ONE dense
#     PE contraction of K = 64ch x 9taps = 576 against a padded,
#     per-token-normalized copy of x ("s"), using two partition-stacked
#     row-shifted copies of s so each matmul contracts 2 taps (K=128).
#   * Per-token LN stats (mean / rstd) via PE matmuls with f32r selector
#     matrices (stats + broadcast), never crossing partitions on DVE.
#   * Gram q@kT via bf16 DMA-xbar transpose of conv outputs + PE matmuls
#     accumulating [128,128] = [[qq,qk],[kq,kk]] per item; channel norms
#     come from the gram diagonal.
#   * softmax on a 64x64 block-masked gram; attn folded with proj into a
#     single [128,128] block-diagonal matrix so the whole output pass is
#     one matmul per 512 tokens + residual fuse.
#
import sys

sys.path.insert(0, "/opt/trn_rl_repo")

import numpy as np

HEADS = 8
C = 64
CH3 = 3 * C


# ----------------------------------------------------------------------------
# host-side parameter prep (per quadrant)
# ----------------------------------------------------------------------------
def _host_params(qkv_w, qkv_b, dw_w, dw_b, ln_w, ln_b, temp, proj_w, proj_b, grw,
                 NST):
    f32 = np.float32
    qkv_w = np.asarray(qkv_w, f32)          # [192, 64]
    dw_w = np.asarray(dw_w, f32)[:, 0]      # [192, 3, 3]
    W_eff = qkv_w * np.asarray(ln_w, f32)[None, :]          # [192, 64]
    b0 = np.asarray(qkv_b, f32) + qkv_w @ np.asarray(ln_b, f32)  # [192]

    # Wb[c, j, ky, kx] = W_eff[c, j] * dw_w[c, ky, kx]
    Wb = W_eff[:, :, None, None] * dw_w[:, None, :, :]      # [192, 64, 3, 3]

    # lhsT for full-K (dy-pair) matmuls: rows (g*64+j), g=0 -> ky=0, g=1 -> ky=1
    lhsTf = np.zeros((128, 2 * 3 * CH3), f32)
    lhsTh = np.zeros((128, 3 * CH3), f32)
    for dxi in range(3):
        for g in range(2):
            # item a: partitions 0-63 hold s(+0) (dy=-1 for row h0 read at
            # offset h0-1), partitions 64-127 hold s(+PW) (dy=0).
            lhsTf[g * 64:(g + 1) * 64, dxi * CH3:(dxi + 1) * CH3] = \
                Wb[:, :, g, dxi].T
            # item b: halves swapped (s(+0) lives on partitions 64-127).
            lhsTf[(1 - g) * 64:(2 - g) * 64,
                  3 * CH3 + dxi * CH3:3 * CH3 + (dxi + 1) * CH3] = \
                Wb[:, :, g, dxi].T
        lhsTh[0:64, dxi * CH3:(dxi + 1) * CH3] = Wb[:, :, 2, dxi].T
        lhsTh[64:128, dxi * CH3:(dxi + 1) * CH3] = Wb[:, :, 2, dxi].T

    wsum = dw_w.sum(axis=(1, 2))                            # [192]
    bias_all = b0 * wsum + np.asarray(dw_b, f32)            # [192]

    # edge corrections (missing taps at image borders)
    top = -b0 * dw_w[:, 0, :].sum(axis=1)
    bot = -b0 * dw_w[:, 2, :].sum(axis=1)
    left = -b0 * dw_w[:, :, 0].sum(axis=1)
    right = -b0 * dw_w[:, :, 2].sum(axis=1)
    c00 = b0 * dw_w[:, 0, 0]
    c01 = b0 * dw_w[:, 0, 2]
    c10 = b0 * dw_w[:, 2, 0]
    c11 = b0 * dw_w[:, 2, 2]
    edges = np.stack([top, bot, left, right, c00, c01, c10, c11], axis=1)  # [192, 8]

    temp_rows = np.repeat(np.asarray(temp, f32), C // HEADS)[:, None]  # [64,1]

    selmu = np.zeros((128, NST * 128), f32)
    selsq = np.zeros((128, NST * 128), f32)
    for c in range(NST):
        selmu[0:64, c * 128 + 2 * c] = 1.0 / C
        selmu[64:128, c * 128 + 2 * c + 1] = 1.0 / C
        selsq[0:64, c * 128 + 64 + 2 * c] = 1.0 / C
        selsq[64:128, c * 128 + 64 + 2 * c + 1] = 1.0 / C
    # per-sub-chunk broadcast selectors (K = 2*NST at base partition 0)
    selb = np.zeros((2 * NST, NST * 128), f32)
    for c in range(NST):
        selb[2 * c, c * 128:c * 128 + 64] = 1.0
        selb[2 * c + 1, c * 128 + 64:(c + 1) * 128] = 1.0

    maskblk = np.full((64, 64), -30000.0, f32)
    for h in range(HEADS):
        maskblk[h * 8:(h + 1) * 8, h * 8:(h + 1) * 8] = 0.0

    diagmask = np.zeros((128, 256), f32)
    for i in range(128):
        diagmask[i, i] = 1.0
        diagmask[i, 128 + i] = 1.0

    eyeT = np.zeros((128, 64), f32)
    eyeT[64:128] = np.eye(64, dtype=f32)

    return {
        "lhsTf": lhsTf.astype(np.float32),
        "lhsTh": lhsTh.astype(np.float32),
        "selmu": selmu,
        "selsq": selsq,
        "selb": selb,
        "projT": np.asarray(proj_w, f32).T.copy(),          # [64(c_in), 64(c_out)]
        "eyeT": eyeT,
        "ones1": np.ones((1, 64), f32),
        "temp_rows": temp_rows,
        "maskblk": maskblk,
        "diagmask": diagmask,
        "bias_qk": bias_all[0:128, None].copy(),
        "bias_v": np.tile(bias_all[128:192, None], (2, 1)).copy(),
        "edge_qk": edges[0:128].copy(),
        "edge_v": np.tile(edges[128:192], (2, 1)).copy(),
        "projb": np.tile(np.asarray(proj_b, f32), 2)[:, None].copy(),  # [128,1]
        "grw": np.full((128, 1), float(np.asarray(grw)), f32),
    }


# ----------------------------------------------------------------------------
# bass program
# ----------------------------------------------------------------------------
def build_nc(Hq, Wq, R, SR, split=True):
    import concourse.bass as bass
    import concourse.mybir as mybir
    from concourse import tile
    from concourse.vector_clock import ScopedClock

    # patch: walrus in this container rejects >1 sync-wait on the tile tail
    # drain; split the waits across individual SP nops instead.
    if not getattr(tile.TileContext, "_ant_drain_patched", False):
        def _patched(self, tick_clock, wait_clock):
            probe = self.nc.sync.nop()
            wait_clock.add_sem_waits(
                probe.ins, ScopedClock({None: tick_clock.global_clock}))
            waits = list(probe.ins.sync_info.on_wait) if probe.ins.sync_info else []
            probe.ins.sync_info = mybir.SyncInfo(on_wait=[], on_update=[])
            for i in range(len(waits)):
                w = self.nc.sync.nop()
                w.ins.sync_info = mybir.SyncInfo(on_wait=waits[i:i + 1], on_update=[])
            self.nc.sync.drain()
            self.nc.all_engine_barrier()
            assert self.sems is not None
            popped = self.nc._tile_sem_poison_stack.pop()
            assert popped is self._sem_poison
            self.nc.clear_and_free_semaphores(list(self.sems.allocated().values()))
            self.nc.all_engine_barrier()
        tile.TileContext._drain_and_barrier = _patched
        tile.TileContext._ant_drain_patched = True

    f32 = mybir.dt.float32
    f32r = mybir.dt.float32r
    bf16 = mybir.dt.bfloat16
    AF = mybir.ActivationFunctionType
    OP = mybir.AluOpType
    AX = mybir.AxisListType

    PW = Wq + 2
    NTOK = Hq * Wq
    NCH = Hq // R            # row-chunks
    NSUB = R // SR           # conv sub-chunks per row-chunk
    SUBT = SR * Wq           # tokens per sub-chunk (<=512, 128-multiple)
    SROWS = R + 2            # s rows per chunk (incl halo)
    NST = SROWS // SR        # stat sub-chunks per row-chunk
    NBLK = (NSUB * SUBT) // 128  # transpose blocks per row-chunk
    NC2 = NTOK // 512 if NTOK % 512 == 0 else None
    OUTT = 512 if NC2 else SUBT
    NOUT = NTOK // OUTT
    assert SUBT <= 512 and (NSUB * SUBT) % 128 == 0 and R % SR == 0
    assert Hq % R == 0 and SROWS % SR == 0

    nc = bass.Bass("TRN2", target_bir_lowering=False, debug=False, num_devices=8)

    def din(name, shape, dt=f32):
        return nc.dram_tensor(name, shape, dt, kind="ExternalInput")

    x_d = din("x", [128, NTOK])
    lhsTf_d = din("lhsTf", [128, 2 * 3 * CH3])
    lhsTh_d = din("lhsTh", [128, 3 * CH3])
    selmu_d = din("selmu", [128, NST * 128])
    selsq_d = din("selsq", [128, NST * 128])
    selb_d = din("selb", [2 * NST, NST * 128])
    projT_d = din("projT", [64, 64])
    eyeT_d = din("eyeT", [128, 64])
    ones1_d = din("ones1", [1, 64])
    temp_d = din("temp_rows", [64, 1])
    maskblk_d = din("maskblk", [64, 64])
    diagmask_d = din("diagmask", [128, 256])
    bias_qk_d = din("bias_qk", [128, 1])
    bias_v_d = din("bias_v", [128, 1])
    edge_qk_d = din("edge_qk", [128, 8])
    edge_v_d = din("edge_v", [128, 8])
    projb_d = din("projb", [128, 1])
    grw_d = din("grw", [128, 1])
    out_d = nc.dram_tensor("out", [128, NTOK], f32, kind="ExternalOutput")

    with tile.TileContext(nc) as tc:
        with (
            tc.tile_pool(name="persist", bufs=1) as pp,
            tc.tile_pool(name="gram_ps", bufs=1, space="PSUM") as gramp,
        ):
            # ---- load params ----
            def ld(d, shape, dt=f32, tag=None):
                t = pp.tile(shape, dt, tag=tag or d.name)
                nc.sync.dma_start(t[:], d[:])
                return t

            selmu = pp.tile([128, NST * 128], bf16, tag="selmu")
            nc.gpsimd.dma_start(selmu[:], selmu_d[:])
            selsq = pp.tile([128, NST * 128], bf16, tag="selsq")
            nc.gpsimd.dma_start(selsq[:], selsq_d[:])
            selb = pp.tile([2 * NST, NST * 128], bf16, tag="selb")
            nc.gpsimd.dma_start(selb[:], selb_d[:])
            eyeT = ld(eyeT_d, [128, 64])
            ones1 = ld(ones1_d, [1, 64])
            temp_rows = ld(temp_d, [64, 1])
            maskblk = ld(maskblk_d, [64, 64])
            diagmask = ld(diagmask_d, [128, 256])
            bias_qk = ld(bias_qk_d, [128, 1])
            bias_v = ld(bias_v_d, [128, 1])
            edge_qk = ld(edge_qk_d, [128, 8])
            edge_v = ld(edge_v_d, [128, 8])
            projb = ld(projb_d, [128, 1])
            grw_t = ld(grw_d, [128, 1])

            # bf16 matmul weights via casting (gpsimd) DMA loads
            lhsTf = pp.tile([128, 2 * 3 * CH3], bf16, tag="lhsTf_bf")
            nc.gpsimd.dma_start(lhsTf[:], lhsTf_d[:])
            lhsTh = pp.tile([128, 3 * CH3], bf16, tag="lhsTh_bf")
            nc.gpsimd.dma_start(lhsTh[:], lhsTh_d[:])
            projT = pp.tile([64, 64], bf16, tag="projT_bf")
            nc.gpsimd.dma_start(projT[:], projT_d[:])

            eps_t = pp.tile([128, 1], f32, tag="eps_t")
            nc.vector.memset(eps_t[:], 1e-5)
            v_pair = pp.tile([128, Hq, Wq], bf16, tag="v_pair")
            CT_pair = pp.tile([128, 128], bf16, tag="CT_pair")
            nc.vector.memset(CT_pair[:], 0.0)
            G_psa = gramp.tile([128, 128], f32, tag="G_psa")
            G_psb = gramp.tile([128, 128], f32, tag="G_psb")

            # ================= PHASE A =================
            from contextlib import ExitStack
            with ExitStack() as es:
                xpool = es.enter_context(tc.tile_pool(name="xc", bufs=2))
                sqpool = es.enter_context(tc.tile_pool(name="sq", bufs=2))
                stpool = es.enter_context(tc.tile_pool(name="stsb", bufs=1))
                t1pool = es.enter_context(tc.tile_pool(name="t1", bufs=2))
                Sapool = es.enter_context(tc.tile_pool(name="Sa", bufs=2))
                Sbpool = es.enter_context(tc.tile_pool(name="Sb", bufs=2))
                accpool = es.enter_context(tc.tile_pool(name="acc", bufs=2))
                qkTpool = es.enter_context(tc.tile_pool(name="qkT", bufs=2))
                stmup = es.enter_context(
                    tc.tile_pool(name="st_mu", bufs=1, space="PSUM"))
                bcmup = es.enter_context(
                    tc.tile_pool(name="bc_mu", bufs=1, space="PSUM"))
                bcrsp = es.enter_context(
                    tc.tile_pool(name="bc_rs", bufs=1, space="PSUM"))
                cqkp = es.enter_context(
                    tc.tile_pool(name="cqk", bufs=2, space="PSUM"))
                cvp = es.enter_context(
                    tc.tile_pool(name="cv", bufs=1, space="PSUM"))
                for ch in range(NCH):
                    r0 = ch * R
                    # ---- x chunk with halo rows [r0-1, r0+R+1) ----
                    xt = xpool.tile([128, SROWS * Wq], f32, tag="xt")
                    if ch == 0:
                        nc.vector.memset(xt[:, 0:Wq], 0.0)
                        nc.sync.dma_start(xt[:, Wq:], x_d[:, 0:(R + 1) * Wq])
                    elif ch == NCH - 1:
                        nc.sync.dma_start(xt[:, 0:(R + 1) * Wq],
                                          x_d[:, (r0 - 1) * Wq:NTOK])
                        nc.vector.memset(xt[:, (R + 1) * Wq:], 0.0)
                    else:
                        nc.sync.dma_start(
                            xt[:], x_d[:, (r0 - 1) * Wq:(r0 + R + 1) * Wq])

                    # ---- per-token LN stats: mu (rows 0..2*NST) and
                    # E[x^2] (rows 64..64+2*NST), one psum tile / group ----
                    xt_bf = sqpool.tile([128, SROWS * Wq], bf16, tag="xt_bf")
                    nc.vector.tensor_copy(xt_bf[:], xt[:])
                    st_ps = stmup.tile([128, SUBT], f32)
                    for c in range(NST):
                        xs = xt_bf[:, c * SUBT:(c + 1) * SUBT]
                        nc.tensor.matmul(
                            st_ps[:], selmu[:, c * 128:(c + 1) * 128],
                            xs, start=(c == 0), stop=False)
                    for c in range(NST):
                        sq_t = sqpool.tile([128, SUBT], bf16, tag="sq_t")
                        nc.scalar.activation(sq_t[:], xt[:, c * SUBT:(c + 1) * SUBT],
                                             AF.Square)
                        nc.tensor.matmul(
                            st_ps[:], selsq[:, c * 128:(c + 1) * 128],
                            sq_t[:], start=False, stop=(c == NST - 1))

                    mu_sb = stpool.tile([2 * NST, SUBT], bf16, tag="mu_sb")
                    nc.scalar.copy(mu_sb[:], st_ps[0:2 * NST, :])
                    mu_f = stpool.tile([2 * NST, SUBT], f32, tag="mu_f")
                    nc.vector.tensor_copy(mu_f[:], st_ps[0:2 * NST, :])
                    var = stpool.tile([2 * NST, SUBT], f32, tag="var")
                    nc.vector.tensor_mul(var[:], mu_f[:], mu_f[:])
                    nc.vector.tensor_sub(var[:], st_ps[64:64 + 2 * NST, :],
                                         var[:])
                    std = stpool.tile([2 * NST, SUBT], f32, tag="std")
                    nc.scalar.activation(std[:], var[:], AF.Sqrt,
                                         bias=eps_t[0:2 * NST, :])
                    rstd_f = stpool.tile([2 * NST, SUBT], f32, tag="rstd_f")
                    nc.vector.reciprocal(rstd_f[:], std[:])
                    rstd_sb = stpool.tile([2 * NST, SUBT], bf16, tag="rstd_sb")
                    nc.vector.tensor_copy(rstd_sb[:], rstd_f[:])

                    # ---- s = (x - mu) * rstd, written padded (PW stride) ----
                    S_a = Sapool.tile([128, SROWS, PW], bf16, tag="S_a")
                    S_b = Sbpool.tile([128, SROWS, PW], bf16, tag="S_b")
                    # zero pad columns on the direct-write halves
                    nc.vector.memset(S_a[0:64, :, 0:1], 0.0)
                    nc.vector.memset(S_a[0:64, :, PW - 1:PW], 0.0)
                    nc.vector.memset(S_b[64:128, :, 0:1], 0.0)
                    nc.vector.memset(S_b[64:128, :, PW - 1:PW], 0.0)
                    for c in range(NST):
                        bc_mu = bcmup.tile([128, SUBT], f32)
                        nc.tensor.matmul(
                            bc_mu[:], selb[:, c * 128:(c + 1) * 128],
                            mu_sb[:], start=True, stop=True)
                        bc_rs = bcrsp.tile([128, SUBT], f32)
                        nc.tensor.matmul(
                            bc_rs[:], selb[:, c * 128:(c + 1) * 128],
                            rstd_sb[:], start=True, stop=True)
                        t1 = t1pool.tile([128, SUBT], f32, tag="t1")
                        nc.vector.tensor_sub(t1[:], xt[:, c * SUBT:(c + 1) * SUBT],
                                             bc_mu[:])
                        sa_v = S_a[0:64, c * SR:(c + 1) * SR, 1:1 + Wq]
                        sb_v = S_b[64:128, c * SR:(c + 1) * SR, 1:1 + Wq]
                        nc.vector.tensor_mul(sa_v, t1[0:64, :], bc_rs[0:64, :])
                        nc.vector.tensor_mul(sb_v, t1[64:128, :], bc_rs[64:128, :])
                    # shifted (+1 row) copies into the opposite halves
                    nc.sync.dma_start(S_a[64:128, 0:SROWS - 1, :],
                                      S_a[0:64, 1:SROWS, :])
                    nc.sync.dma_start(S_b[0:64, 0:SROWS - 1, :],
                                      S_b[64:128, 1:SROWS, :])

                    # ---- fused LN+1x1+dw3x3 conv, per item ----
                    for it in range(2):
                        S = S_a if it == 0 else S_b
                        base = 64 * it
                        acc = accpool.tile([128, NSUB * SUBT], bf16,
                                           tag="acc")
                        for o in range(NSUB):
                            h0 = 1 + o * SR
                            qkps = cqkp.tile([128, SUBT], f32)
                            vps = cvp.tile([128, SUBT], f32)
                            fofs = it * 3 * CH3
                            for (lt, c0) in ((0, 0), (1, 128)):
                                psum = qkps if lt == 0 else vps
                                M = 128 if lt == 0 else 64
                                od = psum[:] if lt == 0 else \
                                    psum[base:base + 64, :]
                                for dxi in range(3):
                                    rhs = S[:, h0 - 1:h0 - 1 + SR, dxi:dxi + Wq]
                                    nc.tensor.matmul(
                                        od, lhsTf[:, fofs + dxi * CH3 + c0:
                                                  fofs + dxi * CH3 + c0 + M],
                                        rhs, start=(dxi == 0), stop=False)
                                for dxi in range(3):
                                    rhs = S[base:base + 64, h0 + 1:h0 + 1 + SR,
                                            dxi:dxi + Wq]
                                    nc.tensor.matmul(
                                        od, lhsTh[base:base + 64,
                                                  dxi * CH3 + c0:
                                                  dxi * CH3 + c0 + M],
                                        rhs, start=False, stop=(dxi == 2))
                            nc.scalar.activation(
                                acc[:, o * SUBT:(o + 1) * SUBT], qkps[:],
                                AF.Identity, bias=bias_qk[:])
                            nc.scalar.activation(
                                v_pair[base:base + 64,
                                       r0 + o * SR:r0 + (o + 1) * SR, :],
                                vps[base:base + 64, :],
                                AF.Identity, bias=bias_v[base:base + 64, :])

                        # ---- border corrections on q,k (and v below) ----
                        a3 = acc[:].rearrange("p (r w) -> p r w", w=Wq)
                        v3 = v_pair[base:base + 64, r0:r0 + R, :]
                        nc.vector.tensor_scalar(a3[:, :, 0:1], a3[:, :, 0:1],
                                                edge_qk[:, 2:3], None, OP.add)
                        nc.vector.tensor_scalar(a3[:, :, Wq - 1:Wq],
                                                a3[:, :, Wq - 1:Wq],
                                                edge_qk[:, 3:4], None, OP.add)
                        nc.vector.tensor_scalar(v3[:, :, 0:1], v3[:, :, 0:1],
                                                edge_v[base:base + 64, 2:3], None, OP.add)
                        nc.vector.tensor_scalar(v3[:, :, Wq - 1:Wq],
                                                v3[:, :, Wq - 1:Wq],
                                                edge_v[base:base + 64, 3:4], None, OP.add)
                        if ch == 0:
                            nc.vector.tensor_scalar(a3[:, 0:1, :], a3[:, 0:1, :],
                                                    edge_qk[:, 0:1], None, OP.add)
                            nc.vector.tensor_scalar(v3[:, 0:1, :], v3[:, 0:1, :],
                                                    edge_v[base:base + 64, 0:1], None, OP.add)
                            nc.vector.tensor_scalar(a3[:, 0:1, 0:1], a3[:, 0:1, 0:1],
                                                    edge_qk[:, 4:5], None, OP.add)
                            nc.vector.tensor_scalar(a3[:, 0:1, Wq - 1:Wq],
                                                    a3[:, 0:1, Wq - 1:Wq],
                                                    edge_qk[:, 5:6], None, OP.add)
                            nc.vector.tensor_scalar(v3[:, 0:1, 0:1], v3[:, 0:1, 0:1],
                                                    edge_v[base:base + 64, 4:5], None, OP.add)
                            nc.vector.tensor_scalar(v3[:, 0:1, Wq - 1:Wq],
                                                    v3[:, 0:1, Wq - 1:Wq],
                                                    edge_v[base:base + 64, 5:6], None, OP.add)
                        if ch == NCH - 1:
                            nc.vector.tensor_scalar(a3[:, R - 1:R, :],
                                                    a3[:, R - 1:R, :],
                                                    edge_qk[:, 1:2], None, OP.add)
                            nc.vector.tensor_scalar(v3[:, R - 1:R, :],
                                                    v3[:, R - 1:R, :],
                                                    edge_v[base:base + 64, 1:2], None, OP.add)
                            nc.vector.tensor_scalar(a3[:, R - 1:R, 0:1],
                                                    a3[:, R - 1:R, 0:1],
                                                    edge_qk[:, 6:7], None, OP.add)
                            nc.vector.tensor_scalar(a3[:, R - 1:R, Wq - 1:Wq],
                                                    a3[:, R - 1:R, Wq - 1:Wq],
                                                    edge_qk[:, 7:8], None, OP.add)
                            nc.vector.tensor_scalar(v3[:, R - 1:R, 0:1],
                                                    v3[:, R - 1:R, 0:1],
                                                    edge_v[base:base + 64, 6:7], None, OP.add)
                            nc.vector.tensor_scalar(v3[:, R - 1:R, Wq - 1:Wq],
                                                    v3[:, R - 1:R, Wq - 1:Wq],
                                                    edge_v[base:base + 64, 7:8], None, OP.add)

                        # ---- transpose + gram accumulate ----
                        qkT = qkTpool.tile([128, NBLK, 128], bf16, tag="qkT")
                        nc.sync.dma_start_transpose(qkT[:], acc[:])
                        G_it = G_psa if it == 0 else G_psb
                        for j in range(NBLK):
                            nc.tensor.matmul(
                                G_it[:], qkT[:, j, :], qkT[:, j, :],
                                start=(ch == 0 and j == 0),
                                stop=(ch == NCH - 1 and j == NBLK - 1))

            # ================= PHASE B =================
            with (
                tc.tile_pool(name="Bsb", bufs=1) as bp,
                tc.tile_pool(name="Bps", bufs=1, space="PSUM") as bpp,
                tc.tile_pool(name="Bps2", bufs=1, space="PSUM") as bpp2,
            ):
                G_sb = bp.tile([128, 256], f32, tag="G_sb")
                nc.scalar.copy(G_sb[:, 0:128], G_psa[:])
                nc.scalar.copy(G_sb[:, 128:256], G_psb[:])
                dm = bp.tile([128, 256], f32, tag="dm")
                nc.vector.tensor_mul(dm[:], G_sb[:], diagmask[:])
                ct_ps = bpp.tile([128, 128], f32)
                for it in range(2):
                    base = 64 * it
                    d_i = bp.tile([128, 1], f32, tag=f"d{it}")
                    nc.vector.reduce_sum(d_i[:], dm[:, it * 128:(it + 1) * 128],
                                         axis=AX.X)
                    nc.scalar.activation(d_i[:], d_i[:], AF.Sqrt, bias=0.0)
                    nc.vector.tensor_scalar_max(d_i[:], d_i[:], 1e-12)
                    inv_i = bp.tile([128, 1], f32, tag=f"inv{it}")
                    nc.vector.reciprocal(inv_i[:], d_i[:])
                    rowfac = bp.tile([64, 1], f32, tag=f"rf{it}")
                    nc.vector.tensor_mul(rowfac[:], inv_i[0:64, :], temp_rows[:])
                    # inv_nk row vector then broadcast down 64 partitions
                    nkrow_ps = bpp2.tile([1, 64], f32)
                    nc.tensor.matmul(nkrow_ps[:], inv_i[64:128, :],
                                     eyeT[64:128, :], start=True, stop=True)
                    nkrow = bp.tile([1, 64], f32, tag=f"nkrow{it}")
                    nc.scalar.copy(nkrow[:], nkrow_ps[:])
                    colfac_ps = bpp2.tile([64, 64], f32)
                    nc.tensor.matmul(colfac_ps[:], ones1[:], nkrow[:],
                                     start=True, stop=True)
                    # masked softmax over the 64x64 q-k block
                    t = bp.tile([64, 64], f32, tag=f"t{it}")
                    Gq = G_sb[0:64, it * 128 + 64:it * 128 + 128]
                    nc.vector.tensor_scalar(t[:], Gq, rowfac[:], None, OP.mult)
                    nc.vector.tensor_mul(t[:], t[:], colfac_ps[:])
                    nc.vector.tensor_add(t[:], t[:], maskblk[:])
                    rmax = bp.tile([64, 1], f32, tag=f"rmax{it}")
                    nc.vector.reduce_max(rmax[:], t[:], axis=AX.X, negate=True)
                    nc.vector.tensor_scalar(t[:], t[:], rmax[:], None, OP.add)
                    nc.scalar.activation(t[:], t[:], AF.Exp)
                    rsum = bp.tile([64, 1], f32, tag=f"rsum{it}")
                    nc.vector.reduce_sum(rsum[:], t[:], axis=AX.X)
                    nc.vector.reciprocal(rsum[:], rsum[:])
                    A_sm = bp.tile([64, 64], bf16, tag=f"A{it}")
                    nc.vector.tensor_scalar(A_sm[:], t[:], rsum[:], None, OP.mult)
                    # C^T = (proj @ A)^T via lhsT=A, rhs=proj^T
                    nc.tensor.matmul(ct_ps[base:base + 64, base:base + 64],
                                     A_sm[:], projT[:], start=True, stop=True)
                    nc.scalar.copy(
                        CT_pair[base:base + 64, base:base + 64],
                        ct_ps[base:base + 64, base:base + 64])

            # ================= PHASE C =================
            v2d = v_pair[:].rearrange("p r w -> p (r w)")
            with (
                tc.tile_pool(name="Cx", bufs=3) as cxp,
                tc.tile_pool(name="Ct", bufs=2) as ctp,
                tc.tile_pool(name="Co", bufs=2) as cop,
                tc.tile_pool(name="Cps", bufs=2, space="PSUM") as cpp,
            ):
                for n in range(NOUT):
                    sl = slice(n * OUTT, (n + 1) * OUTT)
                    xc = cxp.tile([128, OUTT], f32, tag="xc")
                    nc.sync.dma_start(xc[:], x_d[:, sl])
                    ops = cpp.tile([128, OUTT], f32)
                    nc.tensor.matmul(ops[:], CT_pair[:], v2d[:, sl],
                                     start=True, stop=True)
                    t1c = ctp.tile([128, OUTT], f32, tag="t1c")
                    nc.scalar.activation(t1c[:], ops[:], AF.Identity,
                                         bias=projb[:])
                    outc = cop.tile([128, OUTT], f32, tag="outc")
                    nc.vector.scalar_tensor_tensor(
                        outc[:], xc[:], grw_t[:], t1c[:], OP.mult, OP.add)
                    nc.sync.dma_start(out_d[:, sl], outc[:])

    if split:
        _split_waits(nc, mybir, maxw=1)
    return nc


def _split_waits(nc, mybir, maxw=1):
    """The walrus build here rejects instructions carrying more than one
    sync-wait; hoist excess waits onto same-engine NOPs placed before the
    instruction."""
    k = 0
    for f in nc.m.functions:
        for b in f.blocks:
            insts = b.instructions
            out = []
            for inst in insts:
                si = inst.sync_info
                waits = list(si.on_wait) if si is not None else []
                if len(waits) > maxw:
                    keep = waits[-maxw:]
                    excess = waits[:-maxw]
                    for i in range(0, len(excess), maxw):
                        n = mybir.InstNoOp(name=f"wsplit-{k}", ins=[], outs=[])
                        k += 1
                        n.engine = inst.engine
                        n.sync_info = mybir.SyncInfo(
                            on_wait=excess[i:i + maxw], on_update=[])
                        out.append(n)
                    inst.sync_info = mybir.SyncInfo(
                        on_wait=keep, on_update=list(si.on_update))
                out.append(inst)
            b.instructions = out


# ----------------------------------------------------------------------------
# per-core input packing
# ----------------------------------------------------------------------------
def make_in_maps(x, ln_w, ln_b, qkv_w, qkv_b, dw_w, dw_b, temp, proj_w, proj_b,
                 grw, Hq, Wq, NST):
    x = np.asarray(x, np.float32)
    B, Cx, H, W = x.shape
    in_maps = []
    for core in range(8):
        qd, bp = core // 2, core % 2
        rs = slice(0, Hq) if qd < 2 else slice(Hq, 2 * Hq)
        cs = slice(0, Wq) if qd % 2 == 0 else slice(Wq, 2 * Wq)
        xq = np.ascontiguousarray(x[2 * bp:2 * bp + 2, :, rs, cs])
        m = _host_params(qkv_w[qd], qkv_b[qd], dw_w[qd], dw_b[qd],
                         ln_w[qd], ln_b[qd], temp[qd], proj_w[qd],
                         proj_b[qd], grw[qd], NST)
        m = {k: np.ascontiguousarray(v, np.float32) for k, v in m.items()}
        m["x"] = xq.reshape(128, Hq * Wq)
        in_maps.append(m)
    return in_maps


def unpack_out(results, Hq, Wq):
    out = np.zeros((4, C, 2 * Hq, 2 * Wq), np.float32)
    for core in range(8):
        qd, bp = core // 2, core % 2
        rs = slice(0, Hq) if qd < 2 else slice(Hq, 2 * Hq)
        cs = slice(0, Wq) if qd % 2 == 0 else slice(Wq, 2 * Wq)
        o = results[core]["out"].reshape(2, C, Hq, Wq)
        out[2 * bp:2 * bp + 2, :, rs, cs] = o
    return out


# ----------------------------------------------------------------------------
# cached PJRT runner (compile once, execute many)
# ----------------------------------------------------------------------------
_CACHE = {}

HQ, WQ, RR, SRR = 192, 192, 16, 2
_NST = (RR + 2) // SRR


def _get_runner():
    if "run" in _CACHE:
        return _CACHE["run"]
    import jax
    import numpy as _np
    from jax.experimental.shard_map import shard_map
    from jax.sharding import Mesh, PartitionSpec
    import concourse.mybir as mybir
    from concourse import bass2jax

    nc = build_nc(HQ, WQ, RR, SRR)
    bass2jax.install_neuronx_cc_hook()

    n_cores = 8
    partition_name = (nc.partition_id_tensor.name
                      if nc.partition_id_tensor else None)
    in_names, out_names, out_avals, zero_outs = [], [], [], []
    for alloc in nc.m.functions[0].allocations:
        if not isinstance(alloc, mybir.MemoryLocationSet):
            continue
        name = alloc.memorylocations[0].name
        if alloc.kind == "ExternalInput":
            if name != partition_name:
                in_names.append(name)
        elif alloc.kind == "ExternalOutput":
            out_names.append(name)
            shape = tuple(alloc.tensor_shape)
            dtype = mybir.dt.np(alloc.dtype)
            out_avals.append(jax.core.ShapedArray(shape, dtype))
            zero_outs.append(_np.zeros(shape, dtype))
    n_params = len(in_names)
    n_outs = len(out_avals)
    all_names = in_names + out_names
    if partition_name is not None:
        all_names = all_names + [partition_name]
    donate = tuple(range(n_params, n_params + n_outs))

    def _body(*args):
        operands = list(args)
        if partition_name is not None:
            operands.append(bass2jax.partition_id_tensor())
        outs = bass2jax._bass_exec_p.bind(
            *operands,
            out_avals=tuple(out_avals),
            in_names=tuple(all_names),
            out_names=tuple(out_names),
            lowering_input_output_aliases=(),
            sim_require_finite=True,
            sim_require_nnan=True,
            nc=nc,
        )
        return tuple(outs)

    devices = jax.devices()[:n_cores]
    mesh = Mesh(_np.asarray(devices), ("core",))
    sharded = jax.jit(
        shard_map(_body, mesh=mesh,
                  in_specs=(PartitionSpec("core"),) * (n_params + n_outs),
                  out_specs=(PartitionSpec("core"),) * n_outs,
                  check_rep=False),
        donate_argnums=donate, keep_unused=True)

    def run(in_maps):
        concat_in = [
            _np.concatenate([_np.asarray(in_maps[c][k]) for c in range(n_cores)],
                            axis=0)
            for k in in_names
        ]
        concat_zeros = [
            _np.zeros((n_cores * z.shape[0], *z.shape[1:]), z.dtype)
            for z in zero_outs
        ]
        out_arrs = sharded(*concat_in, *concat_zeros)
        return [
            {name: _np.asarray(out_arrs[i]).reshape(n_cores, *out_avals[i].shape)[c]
             for i, name in enumerate(out_names)}
            for c in range(n_cores)
        ]

    _CACHE["run"] = run
    return run


def kernel(x, ln_w, ln_b, qkv_w, qkv_b, dw_w, dw_b, temp, proj_w, proj_b, grw):
    run = _get_runner()
    in_maps = make_in_maps(x, ln_w, ln_b, qkv_w, qkv_b, dw_w, dw_b, temp,
                           proj_w, proj_b, grw, HQ, WQ, _NST)
    results = run(in_maps)
    return unpack_out(results, HQ, WQ).astype(np.float32)
